# revision 13
# baseline (speedup 1.0000x reference)
"""Multi-head attention (B=4, T=2048, C=1024, H=16) on 8 trn2 NeuronCores.

Sharding: core c = 2*b + g handles batch b (of 4) and head-group g (of 2,
8 heads = 512 channels each). Each core computes q/k/v projections for its
512 channels, full TxT attention for its 8 heads, and the partial output
projection out_part = y_local @ Wo[:, g*512:(g+1)*512].T. Host sums the two
partials per batch and adds bo.

Mask trick: keys with mask!=0 contribute exactly 0 to softmax (exp(-inf)),
so the host compacts the key axis to the unmasked positions before the K/V
projections (~halves attention work). The compacted key count is padded to a
multiple of 128; padding lanes get a -1e30 bias fused into the exp.

On-chip layouts (per core):
  xT   [C=1024, T=2048]      x[b][order].T, order = kept keys first; the
                             K/V projections read columns [0:TKP)
  qp2  [128, 2048] x4        q.T packed: tile m holds heads 2m (part 0-63)
                             and 2m+1 (part 64-127)
  kT   [128, TKP] x4         k.T, same head packing
  vaug [TKP, 8*128]          per head 128 cols: 64 ones then 64 v data
  yT   [128, 2048] x4        normalized attention output transposed

Attention runs per head-PAIR: the two heads' score matmuls have K=64
contraction and execute on independent 64-row PE tiles (tile_position
(0,0) / (64,0)) concurrently, writing adjacent halves of one [128,1024]
psum tile that a single exp activation drains. The 64 ones-columns of
vaug replicate the softmax denominator on psum partitions 0-63, so
normalization is reciprocal+multiply straight from psum (no copy, no
partition broadcast).

Schedule: the scalar engine's exp stream (144 x ~1.1us) is the steady-state
governor; the tensor engine total (~178us of streaming) is just below it.
Startup ships x fronts + wv first (V runs under the DMA stream), then wk,
wq so head-pair 0's first score matmul issues as early as possible. All
remaining projection/output work is cut into ~1k-cycle quanta (generators)
that the attention loop pumps between score groups at the exp-deficit rate,
keeping both engines saturated without delaying the next exp.
"""

import numpy as np
import ml_dtypes

import concourse.bass as bass
import concourse.mybir as mybir
import concourse.tile as tile
from concourse import bacc
from concourse.bass_utils import run_bass_kernel_spmd

F32 = mybir.dt.float32
BF16 = mybir.dt.bfloat16
NP_BF16 = ml_dtypes.bfloat16

B, T, C = 4, 2048, 1024
H, D = 16, 64
G = 2                 # head groups (cores per batch)
HL = H // G           # heads per core = 8
DL = HL * D           # local channels = 512
NP = HL // 2          # head pairs per core = 4
SCALE = 1.0 / np.sqrt(D)
NEG = -1e30
N_CORES = 8

_nc_cache: dict = {}


def _dedup_ldweights(nc):
    """Drop Ldweights whose stationary operand, tile size/position and perf
    mode are identical to the immediately preceding (kept) Ldweights — the PE
    array retains its weights across matmuls, so the repeat load is pure
    overhead. Any semaphore waits on a dropped load move to the next matmul."""
    n_rm = 0
    for blk in nc.main_func.blocks:
        insts = blk.instructions
        last_key = None
        pend_waits = []
        drop = []
        for idx in range(len(insts)):
            inst = insts[idx]
            nm = type(inst).__name__
            if nm == "InstLdweights":
                a = inst.ins[0]
                key = (
                    str(getattr(a, "memref", None)),
                    str(getattr(a, "memsetref", None)),
                    a.offset, str(a.ap), str(a.dtype),
                    str(getattr(inst, "tile_size", None)),
                    str(getattr(inst, "tile_position", None)),
                    str(getattr(inst, "perf_mode", None)),
                    str(getattr(inst, "is_transpose", None)),
                )
                si = inst.sync_info
                has_upd = si is not None and len(si.on_update) > 0
                if key == last_key and not has_upd:
                    drop.append(idx)
                    if si is not None and len(si.on_wait) > 0:
                        pend_waits.extend(list(si.on_wait))
                    continue
                last_key = key
            elif nm == "InstMatmult" and pend_waits:
                si = inst.sync_info
                if si is None:
                    inst.sync_info = mybir.SyncInfo(
                        on_wait=list(pend_waits), on_update=[])
                else:
                    si.on_wait = list(si.on_wait) + list(pend_waits)
                pend_waits = []
        for idx in reversed(drop):
            del insts[idx]
        n_rm += len(drop)
    return n_rm


def _build_nc(tkp: int):
    """Build + compile the SPMD Bass program for padded key count tkp."""
    ntk = tkp // 128          # key partition-tiles
    nkc = C // 128            # contraction tiles over C = 8
    nmq = DL // 128           # channel partition-tiles = 4 (== head pairs)
    assert tkp % 128 == 0

    nc = bacc.Bacc(None, num_swdge_queues=2)

    xT_d = nc.dram_tensor("xT", [C, T], BF16, kind="ExternalInput")
    wqT_d = nc.dram_tensor("wqT", [C, DL], BF16, kind="ExternalInput")
    wkT_d = nc.dram_tensor("wkT", [C, DL], BF16, kind="ExternalInput")
    wvT_d = nc.dram_tensor("wvT", [C, DL], BF16, kind="ExternalInput")
    woT_d = nc.dram_tensor("woT", [DL, C], BF16, kind="ExternalInput")
    # bias_all packs [bqp | bkp | bvp | mbp] along the free dim
    nbias = nmq + nmq + DL + ntk
    bias_d = nc.dram_tensor("bias_all", [128, nbias], F32, kind="ExternalInput")
    out_d = nc.dram_tensor("out", [T, C], mybir.dt.float16, kind="ExternalOutput")

    with tile.TileContext(nc) as tc:
        with (
            tc.tile_pool(name="persist", bufs=1) as pp,
            tc.tile_pool(name="work", bufs=4) as wp,
            tc.tile_pool(name="psum", bufs=1, space="PSUM") as psp,
        ):
            # ---- persistent SBUF tensors (wide layout: k-tile k at column k*W) ----
            def persist(shape, dt, tag):
                return pp.tile(shape, dt, tag=tag, name=tag)

            xT_a = persist([128, nkc * T], BF16, "xTa")
            wqT_a = persist([128, nkc * DL], BF16, "wqTa")
            wkT_a = persist([128, nkc * DL], BF16, "wkTa")
            wvT_a = persist([128, nkc * DL], BF16, "wvTa")
            woT_a = persist([128, nmq * C], BF16, "woTa")
            qp_t = [persist([128, T], BF16, f"qp{m}") for m in range(nmq)]
            kT_t = [persist([128, tkp], BF16, f"kT{m}") for m in range(nmq)]
            va_t = [persist([128, HL * 128], BF16, f"va{t}") for t in range(ntk)]
            yT_t = [persist([128, T], BF16, f"yT{m}") for m in range(nmq)]
            bias_t = persist([128, nbias], F32, "bias")
            OQ, OK, OV, OM = 0, nmq, 2 * nmq, 2 * nmq + DL
            scr_t = persist([128, 640], BF16, "scr")  # PE warmup scratch

            # psum slots (8 banks): "s" 2x[128,1024] (4), "y" 2x[128,512] (2),
            # "f" 2x[128,512] (2)
            def psum_tile(shape, tag, name):
                return psp.tile(shape, F32, tag=tag, name=name, bufs=2)

            # ---- input DMAs, in consumption order: x fronts + wv (V runs
            # under the stream), wk, wq, x tails, wo. Fanned over 2 queues. ----
            ENG = [nc.sync, nc.gpsimd]

            def dma(sb, dram, W, k0, k1, eng):
                src = dram[:].rearrange("(k p) n -> p k n", p=128)[:, k0:k1, :]
                dst = sb[:, k0 * W:k1 * W].rearrange("p (k n) -> p k n", n=W)
                eng.dma_start(out=dst, in_=src)

            def dma_cols(sb, dram, W, k, c0, c1, eng):
                src = dram[:].rearrange("(k p) n -> p k n", p=128)[:, k:k + 1, c0:c1]
                dst = sb[:, k * W + c0:k * W + c1].rearrange(
                    "p (k n) -> p k n", n=c1 - c0)
                eng.dma_start(out=dst, in_=src)

            nc.scalar.dma_start(out=bias_t[:], in_=bias_d[:])
            for k in range(nkc):
                dma_cols(xT_a, xT_d, T, k, 0, tkp, ENG[k % 2])
                dma(wvT_a, wvT_d, DL, k, k + 1, ENG[(k + 1) % 2])
            dma(wkT_a, wkT_d, DL, 0, nkc // 2, ENG[0])
            dma(wkT_a, wkT_d, DL, nkc // 2, nkc, ENG[1])
            dma(wqT_a, wqT_d, DL, 0, nkc // 2, ENG[0])
            dma(wqT_a, wqT_d, DL, nkc // 2, nkc, ENG[1])
            if tkp < T:
                for k in range(nkc):
                    dma_cols(xT_a, xT_d, T, k, tkp, T, ENG[k % 2])
            dma(woT_a, woT_d, C, 0, nmq // 2, ENG[0])
            dma(woT_a, woT_d, C, nmq // 2, nmq, ENG[1])

            # ---- PE warmup: trip the HAM clock gate while DMA streams in ----
            nc.vector.memset(scr_t[:], 0.0)
            wps = psp.tile([128, 256], F32, tag="s", name="warmup", bufs=2)
            for w in range(6):
                nc.tensor.matmul(
                    wps[:], lhsT=scr_t[:, 0:128], rhs=scr_t[:, 128:384],
                    start=(w == 0), stop=(w == 5),
                )

            # va ones columns (only cols 0:64 of each head block need init)
            for t in range(ntk):
                nc.vector.memset(
                    va_t[t][:].rearrange("p (h e) -> p h e", e=128)[:, :, 0:64], 1.0)

            uid = [0]
            bv3 = bias_t[:, OV:OV + DL].rearrange("p (h e) -> p h e", e=D)

            def v_add(ps, t):
                dst = va_t[t][:].rearrange("p (h e) -> p h e", e=128)[:, :, 64:128]
                src = ps[:].rearrange("p (h e) -> p h e", e=D)
                nc.vector.tensor_add(dst, src, bv3)

            # ---- startup V: tiles 0..4 k-outer (each arriving xT chunk feeds
            # all five tiles, so the DMA stream never stalls the PE), then
            # tiles 5..ntk-3 back-to-back (all chunks on-chip by then) ----
            v_first = list(range(min(5, ntk)))
            v_mid = list(range(5, max(5, ntk - 2)))
            v_fill = [t for t in range(ntk) if t not in v_first + v_mid]
            vtags = ["s", "s", "y", "y", "f"]
            pss = [psum_tile([128, DL], vtags[i], f"vps{t}")
                   for i, t in enumerate(v_first)]
            for k in range(nkc):
                for ps, t in zip(pss, v_first):
                    nc.tensor.matmul(
                        ps[:],
                        lhsT=xT_a[:, k * T + t * 128:k * T + (t + 1) * 128],
                        rhs=wvT_a[:, k * DL:(k + 1) * DL],
                        start=(k == 0), stop=(k == nkc - 1),
                    )
            for ps, t in zip(pss, v_first):
                v_add(ps, t)
            for i, t in enumerate(v_mid):
                ps = psum_tile([128, DL], ["f", "s"][i % 2], f"vps{t}")
                for k in range(nkc):
                    nc.tensor.matmul(
                        ps[:],
                        lhsT=xT_a[:, k * T + t * 128:k * T + (t + 1) * 128],
                        rhs=wvT_a[:, k * DL:(k + 1) * DL],
                        start=(k == 0), stop=(k == nkc - 1),
                    )
                v_add(ps, t)

            # ---- filler generators: each yield is ~one PE quantum (cycles
            # returned), consumed by pump() inside the attention loop.
            # Emission-order safety: Tile derives dependencies from emission
            # order, so a consumer must never be emitted before its producer.
            # Generators mark what they produced; ensure_* force-drains the
            # (need-ordered) queue up to the required producer. ----
            va_done = set(v_first + v_mid)
            k_done = {}
            q_done = {(0, 0)}

            def gen_v_tile(t, tag="f"):
                uid[0] += 1
                ps = psum_tile([128, DL], tag, f"vps{t}_{uid[0]}")
                for k0 in range(0, nkc, 2):
                    for k in (k0, k0 + 1):
                        nc.tensor.matmul(
                            ps[:],
                            lhsT=xT_a[:, k * T + t * 128:k * T + (t + 1) * 128],
                            rhs=wvT_a[:, k * DL:(k + 1) * DL],
                            start=(k == 0), stop=(k == nkc - 1),
                        )
                    yield 2 * DL
                v_add(ps, t)
                va_done.add(t)
                yield 64

            def gen_k_group(m, chunks, tag="f"):
                uid[0] += 1
                pss = [psum_tile([128, 512], tag, f"kps{uid[0]}_{s0}")
                       for s0, cn in chunks]
                for k in range(nkc):
                    lhsT = wkT_a[:, k * DL + m * 128:k * DL + (m + 1) * 128]
                    for ps, (s0, cn) in zip(pss, chunks):
                        nc.tensor.matmul(
                            ps[:, 0:cn], lhsT=lhsT,
                            rhs=xT_a[:, k * T + s0:k * T + s0 + cn],
                            start=(k == 0), stop=(k == nkc - 1),
                        )
                    yield sum(cn for _, cn in chunks)
                for ps, (s0, cn) in zip(pss, chunks):
                    nc.vector.tensor_scalar_add(
                        kT_t[m][:, s0:s0 + cn], ps[:, 0:cn],
                        bias_t[:, OK + m:OK + m + 1])
                k_done[m] = k_done.get(m, 0) + 1
                yield 64

            def gen_q_group(m, ns, tag="f"):
                uid[0] += 1
                pss = [psum_tile([128, 512], tag, f"qps{uid[0]}_{n}")
                       for n in ns]
                for k in range(nkc):
                    lhsT = wqT_a[:, k * DL + m * 128:k * DL + (m + 1) * 128]
                    for ps, n in zip(pss, ns):
                        nc.tensor.matmul(
                            ps[:], lhsT=lhsT,
                            rhs=xT_a[:, k * T + n * 512:k * T + (n + 1) * 512],
                            start=(k == 0), stop=(k == nkc - 1),
                        )
                    yield 512 * len(ns)
                for ps, n in zip(pss, ns):
                    nc.vector.tensor_scalar_add(
                        qp_t[m][:, n * 512:(n + 1) * 512], ps[:],
                        bias_t[:, OQ + m:OQ + m + 1])
                q_done.update((m, n) for n in ns)
                yield 64

            OENG = [nc.sync, nc.gpsimd]
            oq = [0]

            def gen_o_unit(mt, tag="f"):
                uid[0] += 1
                pss = [psum_tile([128, 512], tag, f"ops{uid[0]}_{h2}")
                       for h2 in range(2)]
                for kt in range(nmq):
                    lhsT = yT_t[kt][:, mt * 128:(mt + 1) * 128]
                    for h2 in range(2):
                        nc.tensor.matmul(
                            pss[h2][:], lhsT=lhsT,
                            rhs=woT_a[:, kt * C + h2 * 512:kt * C + (h2 + 1) * 512],
                            start=(kt == 0), stop=(kt == nmq - 1),
                        )
                    yield 1024
                for h2 in range(2):
                    o_sb = wp.tile([128, 512], mybir.dt.float16, tag="o",
                                   name=f"osb{uid[0]}_{h2}", bufs=4)
                    nc.vector.tensor_copy(o_sb[:], pss[h2][:])
                    eng = OENG[oq[0] % 2]
                    oq[0] += 1
                    eng.dma_start(
                        out=out_d[mt * 128:(mt + 1) * 128,
                                  h2 * 512:(h2 + 1) * 512],
                        in_=o_sb[:])
                    yield 64

            # k chunks grouped in lhsT-sharing pairs
            k_chunks = [(s0, min(512, tkp - s0)) for s0 in range(0, tkp, 512)]
            k_groups = [k_chunks[i:i + 2] for i in range(0, len(k_chunks), 2)]

            # ---- startup K (pair 0) + Q (pair 0, chunk 0): the gate for the
            # first score matmul ----
            ktags = [["y", "y"], ["f", "s"], ["s", "y"]]
            for g, tags in zip(k_groups, ktags):
                pssk = [psum_tile([128, 512], tg, f"kps0_{s0}")
                        for (s0, cn), tg in zip(g, tags)]
                for k in range(nkc):
                    lhsT = wkT_a[:, k * DL:k * DL + 128]
                    for ps, (s0, cn) in zip(pssk, g):
                        nc.tensor.matmul(
                            ps[:, 0:cn], lhsT=lhsT,
                            rhs=xT_a[:, k * T + s0:k * T + s0 + cn],
                            start=(k == 0), stop=(k == nkc - 1),
                        )
                for ps, (s0, cn) in zip(pssk, g):
                    nc.vector.tensor_scalar_add(
                        kT_t[0][:, s0:s0 + cn], ps[:, 0:cn],
                        bias_t[:, OK:OK + 1])
            psq = psum_tile([128, 512], "s", "qps00")
            for k in range(nkc):
                nc.tensor.matmul(
                    psq[:], lhsT=wqT_a[:, k * DL:k * DL + 128],
                    rhs=xT_a[:, k * T:k * T + 512],
                    start=(k == 0), stop=(k == nkc - 1),
                )
            nc.vector.tensor_scalar_add(
                qp_t[0][:, 0:512], psq[:], bias_t[:, OQ:OQ + 1])
            k_done[0] = len(k_groups)

            # ---- filler queue in need-order: v tail tiles (PV of the first
            # attention), then per-pair k+q(chunk 0) for qc=0, then q chunk n
            # just before qc=n, with o units for qc-1 appended at each qc ----
            gens = []
            for t in v_fill:
                gens.append(gen_v_tile(t))
            for m in range(1, nmq):
                for g in k_groups:
                    gens.append(gen_k_group(m, g))
                gens.append(gen_q_group(m, [0]))

            def pump(cycles):
                while cycles > 0 and gens:
                    try:
                        cycles -= next(gens[0])
                    except StopIteration:
                        gens.pop(0)

            def force_step():
                try:
                    next(gens[0])
                except StopIteration:
                    gens.pop(0)

            def ensure_va(t):
                while t not in va_done and gens:
                    force_step()

            def ensure_mq(m, qc):
                while gens and (k_done.get(m, 0) < len(k_groups)
                                or (m, qc) not in q_done):
                    force_step()

            EXPF = mybir.ActivationFunctionType.Exp
            BUDGET = 2400   # ~exp-pace deficit per 2-tile score group, cycles

            def attention(m, qc):
                """Head pair m (heads 2m, 2m+1), query chunk qc (512 wide)."""
                q0 = qc * 512
                ensure_mq(m, qc)
                uid[0] += 1
                yps = psum_tile([128, 512], "y", f"yps{uid[0]}")
                yps2 = psum_tile([128, 512], "y", f"yps2_{uid[0]}")
                pend = []  # software-pipelined PV: lag one t-step behind exp

                def pv(t, p_sb):
                    ensure_va(t)
                    nc.tensor.matmul(
                        yps[:],
                        lhsT=va_t[t][:, (2 * m) * 128:(2 * m + 1) * 128],
                        rhs=p_sb[:, 0:512],
                        start=(t == 0), stop=(t == ntk - 1),
                    )
                    nc.tensor.matmul(
                        yps2[:],
                        lhsT=va_t[t][:, (2 * m + 1) * 128:(2 * m + 2) * 128],
                        rhs=p_sb[:, 512:1024],
                        start=(t == 0), stop=(t == ntk - 1),
                    )

                def s_mm(t):
                    # two K=64 matmuls on independent 64-row PE tiles
                    uid[0] += 1
                    s_ps = psum_tile([128, 1024], "s", f"sps{uid[0]}")
                    nc.tensor.matmul(
                        s_ps[:, 0:512],
                        lhsT=kT_t[m][0:64, t * 128:(t + 1) * 128],
                        rhs=qp_t[m][0:64, q0:q0 + 512],
                        start=True, stop=True,
                    )
                    nc.tensor.matmul(
                        s_ps[:, 512:1024],
                        lhsT=kT_t[m][64:128, t * 128:(t + 1) * 128],
                        rhs=qp_t[m][64:128, q0:q0 + 512],
                        start=True, stop=True,
                    )
                    return s_ps

                # S matmuls batched two t-steps at a time: the 64-row loads of
                # step t+1 overlap the streaming of step t's opposite tile
                for t0 in range(0, ntk, 2):
                    ts = [t for t in (t0, t0 + 1) if t < ntk]
                    sps = [s_mm(t) for t in ts]
                    for t, s_ps in zip(ts, sps):
                        p_sb = wp.tile([128, 1024], BF16, tag="p",
                                       name=f"p{uid[0]}_{t}", bufs=8)
                        nc.scalar.activation(
                            p_sb[:], s_ps[:], EXPF,
                            bias=bias_t[:, OM + t:OM + t + 1], scale=float(SCALE),
                        )
                        pend.append((t, p_sb))
                    while len(pend) > 2:
                        pv(*pend.pop(0))
                    pump(BUDGET)
                while pend:
                    pv(*pend.pop(0))
                # normalize straight from psum: partitions 0-63 hold the
                # denominator (ones-columns), 64-127 the numerator
                uid[0] += 1
                for hp, ps in ((0, yps), (1, yps2)):
                    rec = wp.tile([128, 512], F32, tag="rec",
                                  name=f"rec{uid[0]}_{hp}", bufs=2)
                    nc.vector.reciprocal_approx_fast(rec[0:64, :], ps[0:64, :])
                    nc.vector.tensor_mul(
                        yT_t[m][64 * hp:64 * hp + 64, q0:q0 + 512],
                        ps[64:128, :], rec[0:64, :],
                    )

            for qc in range(T // 512):
                # queue next chunk's q projections ahead of need, then the
                # output rows that became complete when qc-1 finished
                if qc + 1 < T // 512:
                    for m in range(nmq):
                        gens.append(gen_q_group(m, [qc + 1]))
                if qc >= 1:
                    gens.extend(gen_o_unit(mt)
                                for mt in range(4 * (qc - 1), 4 * qc))
                for m in range(NP):
                    attention(m, qc)

            # drain leftovers, then the final output rows (only available
            # after the last attention) on rotating psum tags to pipeline
            pump(1 << 30)
            gens.extend(gen_o_unit(mt, tag)
                        for mt, tag in zip(range(3 * (T // 512), T // 128),
                                           ["s", "y", "f", "s"]))
            pump(1 << 30)

    _dedup_ldweights(nc)
    nc.compile()
    return nc


def _get_nc(tkp: int):
    if tkp not in _nc_cache:
        _nc_cache[tkp] = _build_nc(tkp)
    return _nc_cache[tkp]


def kernel(x, mask, Wk, bk, Wq, bq, Wv, bv, Wo, bo, _run_kwargs=None):
    x = np.asarray(x, dtype=np.float32)
    mask = np.asarray(mask)
    Wk, bk = np.asarray(Wk, np.float32), np.asarray(bk, np.float32)
    Wq, bq = np.asarray(Wq, np.float32), np.asarray(bq, np.float32)
    Wv, bv = np.asarray(Wv, np.float32), np.asarray(bv, np.float32)
    Wo, bo = np.asarray(Wo, np.float32), np.asarray(bo, np.float32)

    keep = [np.flatnonzero(mask[b] == 0) for b in range(B)]
    max_keep = max(len(kp) for kp in keep)
    tkp = max(128, -(-max_keep // 128) * 128)
    ntk = tkp // 128
    nmq = DL // 128

    nc = _get_nc(tkp)

    in_maps = []
    orders = []
    for b in range(B):
        # kept-key positions first: the device reads keys as xT[:, :tkp]
        order = np.concatenate([keep[b], np.flatnonzero(mask[b] != 0)])
        orders.append(order)
        xT = np.ascontiguousarray(x[b][order].T).astype(NP_BF16)
        mb = np.zeros(tkp, np.float32)
        mb[len(keep[b]):] = NEG
        mbp = np.ascontiguousarray(mb.reshape(ntk, 128).T)
        for g in range(G):
            gs, ge = g * DL, (g + 1) * DL
            bias_all = np.concatenate([
                bq[gs:ge].reshape(nmq, 128).T,
                bk[gs:ge].reshape(nmq, 128).T,
                np.broadcast_to(bv[gs:ge], (128, DL)),
                mbp,
            ], axis=1).astype(np.float32)
            in_maps.append({
                "xT": xT,
                "wqT": np.ascontiguousarray(Wq[gs:ge].T).astype(NP_BF16),
                "wkT": np.ascontiguousarray(Wk[gs:ge].T).astype(NP_BF16),
                "wvT": np.ascontiguousarray(Wv[gs:ge].T).astype(NP_BF16),
                "woT": np.ascontiguousarray(Wo[:, gs:ge].T).astype(NP_BF16),
                "bias_all": np.ascontiguousarray(bias_all),
            })

    kw = _run_kwargs or {}
    res = run_bass_kernel_spmd(nc, in_maps, list(range(N_CORES)), **kw)

    out = np.empty((B, T, C), np.float32)
    for b in range(B):
        summed = (res.results[2 * b]["out"].astype(np.float32)
                  + res.results[2 * b + 1]["out"].astype(np.float32) + bo)
        out[b][orders[b]] = summed  # undo the query-position permutation
    if kw:
        kernel.last_result = res
    return out
